# revision 1
# baseline (speedup 1.0000x reference)
"""ClusterAttn Trainium2 kernel (Bass/Tile), 8-way data parallel over batch.

Full inputs in, full outputs out. Internally:
  - batch B=32 is split 4-per-core across 8 NeuronCores (pure DP).
  - BN1 is folded into cluster_weights (+W_ga appended) on the host;
    BN2 folded into a per-cluster scale + bias matrix; Wq pre-transposed.
  - attention is re-associated: scores = x @ (Wq @ k^T), out = attn @ (v @ Wp2).
  - all matmuls run as float32r (full PE rate at moving-dim >= 256).
"""

from contextlib import ExitStack

import numpy as np

import concourse.bass as bass
import concourse.bacc as bacc
import concourse.tile as tile
import concourse.mybir as mybir
from concourse import bass_utils
from concourse.masks import make_identity

dt = mybir.dt
AF = mybir.ActivationFunctionType
ALU = mybir.AluOpType

EPS = 1e-5
N_CORES = 8
B, S, D = 32, 1024, 768
E, G, C, P = 2, 8, 64, 384
EF = E * D            # 1536
GC = G * C            # 512
GFS = EF // G         # 192
NB = B // N_CORES     # batches per core
NT = S // 128         # token tiles per batch
F32 = dt.float32
F32R = dt.float32r


def _r(ap):
    """View an fp32 AP as float32r for the tensor engine."""
    return ap.bitcast(F32R)


def _f(ap):
    """View a float32r AP as fp32 for vector/scalar engines."""
    return ap.bitcast(F32)


def build_program(flags):
    has_bexp, has_bq, has_bkv, has_bp2 = flags
    nc = bacc.Bacc(
        "TRN2",
        debug=False,
        enable_asserts=False,
        num_devices=N_CORES,
    )

    x_d = nc.dram_tensor("x", (NB, S, D), F32R, kind="ExternalInput").ap()
    out_d = nc.dram_tensor("out", (NB, S, D), F32, kind="ExternalOutput").ap()
    wexp_d = nc.dram_tensor("wexp", (D, EF), F32R, kind="ExternalInput").ap()
    cwf_d = nc.dram_tensor("cwf", (EF, GC + G), F32R, kind="ExternalInput").ap()
    bias1_d = nc.dram_tensor("bias1", (GC + G,), F32, kind="ExternalInput").ap()
    wproj_d = nc.dram_tensor("wproj", (GFS, D), F32R, kind="ExternalInput").ap()
    s2_d = nc.dram_tensor("s2", (C, 1), F32, kind="ExternalInput").ap()
    bias2_d = nc.dram_tensor("bias2", (C, D), F32, kind="ExternalInput").ap()
    wkv_d = nc.dram_tensor("wkv", (D, 2 * P), F32R, kind="ExternalInput").ap()
    wqT_d = nc.dram_tensor("wqT", (P, D), F32R, kind="ExternalInput").ap()
    wp2_d = nc.dram_tensor("wp2", (P, D), F32R, kind="ExternalInput").ap()
    bexp_d = bq_d = bkv_d = bp2_d = None
    if has_bexp:
        bexp_d = nc.dram_tensor("bexp", (EF,), F32, kind="ExternalInput").ap()
    if has_bq:
        bq_d = nc.dram_tensor("bqT", (P, 1), F32R, kind="ExternalInput").ap()
    if has_bkv:
        bkv_d = nc.dram_tensor("bkv", (2 * P,), F32, kind="ExternalInput").ap()
    if has_bp2:
        bp2_d = nc.dram_tensor("bp2", (D,), F32, kind="ExternalInput").ap()

    with tile.TileContext(nc) as tc, ExitStack() as ctx:
        # ---------------- pools ----------------
        const = ctx.enter_context(tc.tile_pool(name="const", bufs=1))
        mid = ctx.enter_context(tc.tile_pool(name="mid", bufs=1))
        p_x = ctx.enter_context(tc.tile_pool(name="p_x", bufs=3))
        p_xt = ctx.enter_context(tc.tile_pool(name="p_xt", bufs=3))
        p_fea = ctx.enter_context(tc.tile_pool(name="p_fea", bufs=3))
        p_feat = ctx.enter_context(tc.tile_pool(name="p_feat", bufs=3))
        p_act = ctx.enter_context(tc.tile_pool(name="p_act", bufs=2))
        p_sm = ctx.enter_context(tc.tile_pool(name="p_sm", bufs=4))
        p_out = ctx.enter_context(tc.tile_pool(name="p_out", bufs=2))
        p_cacc = ctx.enter_context(tc.tile_pool(name="p_cacc", bufs=2))
        ps_t = ctx.enter_context(tc.tile_pool(name="ps_t", bufs=3, space="PSUM"))
        ps_mm = ctx.enter_context(tc.tile_pool(name="ps_mm", bufs=3, space="PSUM"))
        ps_c = ctx.enter_context(tc.tile_pool(name="ps_c", bufs=2, space="PSUM"))

        # ---------------- constants ----------------
        # (x prefetch is emitted first, inside the emission section below,
        #  via forward-declared pool; DMA order follows program order.)
        x_pref = {}
        for _pt in range(2):
            _xt = p_xt.tile([128, 6, 128], F32R, tag="xt", name=f"xpf0_{_pt}")
            for _ko in range(6):
                nc.sync.dma_start(
                    _xt[:, _ko, :],
                    x_d[0, _pt * 128:(_pt + 1) * 128,
                        _ko * 128:(_ko + 1) * 128].rearrange("s ki -> ki s"))
            x_pref[(0, _pt)] = _xt

        ident_f = const.tile([128, 128], F32)
        make_identity(nc, ident_f[:])
        ident = const.tile([128, 128], F32R)
        nc.vector.tensor_copy(ident[:], ident_f[:])

        wexp_sb = const.tile([128, 6, EF], F32R)
        wexp_r = wexp_d.rearrange("(ko ki) n -> ki ko n", ki=128)
        cwf_sb = const.tile([128, 12, GC + G], F32R)
        cwf_r = cwf_d.rearrange("(ko ki) n -> ki ko n", ki=128)
        nc.sync.dma_start(wexp_sb[:, :, 0:512], wexp_r[:, :, 0:512])
        nc.sync.dma_start(cwf_sb[:, :, 0:260], cwf_r[:, :, 0:260])
        bias1_sb = const.tile([128, GC + G], F32)
        nc.gpsimd.dma_start(bias1_sb[:], bias1_d.partition_broadcast(128))
        wproj_sb = const.tile([128, 2, D], F32R)
        s2_sb = const.tile([C, 1], F32)
        nc.sync.dma_start(s2_sb[:], s2_d)
        bias2_sb = const.tile([C, D], F32)
        nc.sync.dma_start(bias2_sb[:], bias2_d)
        wkv_sb = const.tile([128, 6, 2 * P], F32R)
        wqT_sb = const.tile([128, 3, D], F32R)
        wp2_sb = const.tile([128, 3, D], F32R)

        nc.sync.dma_start(wexp_sb[:, :, 512:1024], wexp_r[:, :, 512:1024])
        nc.sync.dma_start(cwf_sb[:, :, 260:520], cwf_r[:, :, 260:520])
        nc.sync.dma_start(wexp_sb[:, :, 1024:1536], wexp_r[:, :, 1024:1536])
        nc.sync.dma_start(wproj_sb[:, 0, :], wproj_d[0:128, :])
        nc.sync.dma_start(wproj_sb[0:64, 1, :], wproj_d[128:GFS, :])
        nc.sync.dma_start(wkv_sb[:], wkv_d.rearrange("(ko ki) n -> ki ko n", ki=128))
        nc.sync.dma_start(wqT_sb[:], wqT_d.rearrange("(ko ki) n -> ki ko n", ki=128))
        nc.sync.dma_start(wp2_sb[:], wp2_d.rearrange("(ko ki) n -> ki ko n", ki=128))
        if has_bexp:
            bexp_sb = const.tile([128, EF], F32)
            nc.gpsimd.dma_start(bexp_sb[:], bexp_d.partition_broadcast(128))
        if has_bq:
            bq_sb = const.tile([128, 3, 1], F32R)
            nc.sync.dma_start(bq_sb[:], bq_d.rearrange("(ko ki) n -> ki ko n", ki=128))
            ones_sb = const.tile([1, 128], F32R)
            nc.vector.memset(ones_sb[:], 1.0)
        if has_bkv:
            bkv_sb = const.tile([C, 2 * P], F32)
            nc.gpsimd.dma_start(bkv_sb[:], bkv_d.partition_broadcast(C))
        if has_bp2:
            bp2_sb = const.tile([128, D], F32)
            nc.gpsimd.dma_start(bp2_sb[:], bp2_d.partition_broadcast(128))

        def transpose_to(out_ps, in_ap, start=True, stop=True):
            """PE transpose of fp32 in_ap -> float32r psum tile slice."""
            kp = in_ap.partition_size()
            nc.tensor.matmul(
                out_ps,
                _r(in_ap),
                _r(ident[0:kp, 0:kp]),
                is_transpose=True,
                start=start,
                stop=stop,
                skip_group_check=True,
            )

        def load_xt(b, t):
            """DMA one x token-tile directly in transposed [128 d, 6, 128 s]
            layout (d-contiguous 512B bursts)."""
            if (b, t) in x_pref:
                return x_pref.pop((b, t))
            xt = p_xt.tile([128, 6, 128], F32R, tag="xt")
            for ko in range(6):
                nc.sync.dma_start(
                    xt[:, ko, :],
                    x_d[b, t * 128:(t + 1) * 128,
                        ko * 128:(ko + 1) * 128].rearrange("s ki -> ki s"))
            return xt

        cent_store = {}
        seg = {}
        inv_sqrt_p = float(1.0 / np.sqrt(np.float32(P)))

        def seg_X(b, t):
            if b not in cent_store:
                cent_store[b] = ps_c.tile([128, 384], F32, tag="cent",
                                          name=f"centps{b}")
            seg[(b, t)] = {"xt": load_xt(b, t)}

        def seg_F(b, t):
            xt = seg[(b, t)]["xt"]
            if True:

                # fea = x @ W_exp (+ b_exp)   [128 tok, 1536]
                fea = p_fea.tile([128, EF], F32R, tag="fea")
                for n3 in range(3):
                    fp = ps_mm.tile([128, 512], F32, tag="mm")
                    for k in range(6):
                        nc.tensor.matmul(
                            fp[:], _r(xt[:, k, :]),
                            _r(wexp_sb[:, k, n3 * 512:(n3 + 1) * 512]),
                            start=(k == 0), stop=(k == 5))
                    dst = fea[:, n3 * 512:(n3 + 1) * 512]
                    if has_bexp:
                        nc.vector.tensor_add(dst, fp[:], bexp_sb[:, n3 * 512:(n3 + 1) * 512])
                    else:
                        nc.scalar.copy(dst, fp[:])
                seg[(b, t)]["fea"] = fea

        def seg_T(b, t):
            fea = seg[(b, t)]["fea"]
            if True:
                # feaT [128 ef, 12, 128 tok]
                feat = p_feat.tile([128, 12, 128], F32R, tag="feat")
                for grp in range(3):
                    tp = ps_t.tile([128, 512], F32R, tag="t")
                    for i in range(4):
                        m = grp * 4 + i
                        transpose_to(tp[:, i * 128:(i + 1) * 128],
                                     fea[:, m * 128:(m + 1) * 128],
                                     start=(i == 0), stop=(i == 3))
                    nc.vector.tensor_copy(
                        feat[:, grp * 4:(grp + 1) * 4, :].rearrange("p a b -> p (a b)"),
                        _f(tp[:]))
                seg[(b, t)]["feat"] = feat

        def seg_A(b, t):
            feat = seg[(b, t)]["feat"]
            if True:
                # act_pre = fea @ cw_f + bias1   [128 tok, 520]
                act = p_act.tile([128, GC + G], F32, tag="act")
                for n2 in range(2):
                    apm = ps_mm.tile([128, 260], F32, tag="mm")
                    for k2 in range(12):
                        nc.tensor.matmul(
                            apm[:], _r(feat[:, k2, :]),
                            _r(cwf_sb[:, k2, n2 * 260:(n2 + 1) * 260]),
                            start=(k2 == 0), stop=(k2 == 11))
                    nc.vector.tensor_add(act[:, n2 * 260:(n2 + 1) * 260], apm[:],
                                         bias1_sb[:, n2 * 260:(n2 + 1) * 260])

                # grouped softmax * sigmoid gate
                e = p_act.tile([128, GC], F32, tag="e")
                nc.scalar.activation(e[:], act[:, 0:GC], AF.Exp)
                ssum = p_sm.tile([128, G], F32, tag="ssum")
                nc.vector.reduce_sum(ssum[:], e[:].rearrange("p (g c) -> p g c", g=G),
                                     axis=mybir.AxisListType.X)
                eneg = p_sm.tile([128, G], F32, tag="eneg")
                nc.scalar.activation(eneg[:], act[:, GC:GC + G], AF.Exp, scale=-1.0)
                nc.vector.tensor_scalar_add(eneg[:], eneg[:], 1.0)
                ga = p_sm.tile([128, G], F32, tag="ga")
                nc.vector.reciprocal(ga[:], eneg[:])
                rs = p_sm.tile([128, G], F32, tag="rs")
                nc.vector.reciprocal(rs[:], ssum[:])
                nc.vector.tensor_mul(rs[:], rs[:], ga[:])
                actf = p_act.tile([128, GC], F32R, tag="actf")
                nc.vector.tensor_tensor(
                    out=actf[:].rearrange("p (g c) -> p g c", g=G),
                    in0=e[:].rearrange("p (g c) -> p g c", g=G),
                    in1=rs[:].unsqueeze(2).broadcast_to((128, G, C)),
                    op=ALU.mult)
                seg[(b, t)]["actf"] = actf

        def seg_C(b, t):
            st = seg.pop((b, t))
            fea, actf = st["fea"], st["actf"]
            del st
            cp = cent_store[b]
            if True:
                # cent accumulation: all 4 diagonal blocks sum into ONE psum
                # bank; the two extracted sub-blocks never overlap, so psum
                # accumulation performs the over-mi reduction for free.
                for mi in range(4):
                    nc.tensor.matmul(
                        cp[:], _r(actf[:, mi * 128:(mi + 1) * 128]),
                        _r(fea[:, mi * 384:(mi + 1) * 384]),
                        start=(t == 0 and mi == 0),
                        stop=(t == NT - 1 and mi == 3),
                        skip_group_check=True)

        def mid_phase(b):
            cp = cent_store.pop(b)
            cent = p_cacc.tile([C, GFS], F32R, tag="cacc")
            nc.vector.tensor_copy(cent[:], cp[0:64, 0:192])
            nc.vector.tensor_add(cent[:], cent[:], cp[64:128, 192:384])

            # centT [192, 64] as [128, 2, 64]
            centT = mid.tile([128, 2, C], F32R, tag="centT")
            ctp = ps_t.tile([128, 512], F32R, tag="t")
            transpose_to(ctp[:, 0:64], cent[:, 0:128])
            transpose_to(ctp[0:64, 64:128], cent[:, 128:192])
            nc.vector.tensor_copy(centT[:, 0, :], _f(ctp[:, 0:64]))
            nc.vector.tensor_copy(centT[0:64, 1, :], _f(ctp[0:64, 64:128]))

            # nc2 = BN2(cent @ W_proj + b_proj)  [64, 768]
            nc2 = mid.tile([C, D], F32R, tag="nc2")
            for ng, (n0, nn) in enumerate(((0, 512), (512, 256))):
                np_ps = ps_mm.tile([128, 512], F32, tag="mm")
                nc.tensor.matmul(np_ps[0:C, 0:nn], _r(centT[:, 0, :]),
                                 _r(wproj_sb[:, 0, n0:n0 + nn]), start=True, stop=False)
                nc.tensor.matmul(np_ps[0:C, 0:nn], _r(centT[0:64, 1, :]),
                                 _r(wproj_sb[0:64, 1, n0:n0 + nn]), start=False, stop=True)
                nc.vector.scalar_tensor_tensor(
                    out=nc2[:, n0:n0 + nn], in0=np_ps[0:C, 0:nn], scalar=s2_sb[:, 0:1],
                    in1=bias2_sb[:, n0:n0 + nn], op0=ALU.mult, op1=ALU.add)

            # nc2T [768, 64] as [128, 6, 64]
            nc2T = mid.tile([128, 6, C], F32R, tag="nc2T")
            for grp in range(2):
                ntp = ps_t.tile([128, 512], F32R, tag="t")
                for i in range(3):
                    transpose_to(ntp[:, i * 64:(i + 1) * 64],
                                 nc2[:, (grp * 3 + i) * 128:(grp * 3 + i + 1) * 128],
                                 start=(i == 0), stop=(i == 2))
                nc.vector.tensor_copy(
                    nc2T[:, grp * 3:(grp + 1) * 3, :].rearrange("p a b -> p (a b)"),
                    _f(ntp[:, 0:192]))

            # kv = nc2 @ Wkv (+ bkv)   [64, 768]
            kv = mid.tile([C, 2 * P], F32R, tag="kv")
            for ng, (n0, nn) in enumerate(((0, 512), (512, 256))):
                kv_ps = ps_mm.tile([128, 512], F32, tag="mm")
                for k in range(6):
                    nc.tensor.matmul(kv_ps[0:C, 0:nn], _r(nc2T[:, k, :]),
                                     _r(wkv_sb[:, k, n0:n0 + nn]),
                                     start=(k == 0), stop=(k == 5))
                if has_bkv:
                    nc.vector.tensor_add(kv[:, n0:n0 + nn], kv_ps[0:C, 0:nn],
                                         bkv_sb[:, n0:n0 + nn])
                else:
                    nc.scalar.copy(kv[:, n0:n0 + nn], kv_ps[0:C, 0:nn])

            # kT, vT  [128, 3, 64]
            kT = mid.tile([128, 3, C], F32R, tag="kT")
            vT = mid.tile([128, 3, C], F32R, tag="vT")
            ktp = ps_t.tile([128, 512], F32R, tag="t")
            for i in range(3):
                transpose_to(ktp[:, i * 64:(i + 1) * 64], kv[:, i * 128:(i + 1) * 128],
                             start=(i == 0), stop=(i == 2))
            nc.vector.tensor_copy(kT[:].rearrange("p a b -> p (a b)"), _f(ktp[:, 0:192]))
            vtp = ps_t.tile([128, 512], F32R, tag="t")
            for i in range(3):
                transpose_to(vtp[:, i * 64:(i + 1) * 64],
                             kv[:, P + i * 128:P + (i + 1) * 128],
                             start=(i == 0), stop=(i == 2))
            nc.vector.tensor_copy(vT[:].rearrange("p a b -> p (a b)"), _f(vtp[:, 0:192]))

            # wqk = Wq @ k^T  [768, 64] as [128, 6, 64]
            wqk = mid.tile([128, 6, C], F32R, tag="wqk")
            for m in range(6):
                wq_ps = ps_t.tile([128, 512], F32, tag="t")
                for k3 in range(3):
                    nc.tensor.matmul(wq_ps[:, 0:C], _r(wqT_sb[:, k3, m * 128:(m + 1) * 128]),
                                     _r(kT[:, k3, :]), start=(k3 == 0), stop=(k3 == 2))
                nc.scalar.copy(wqk[:, m, :], wq_ps[:, 0:C])
            if has_bq:
                bc_ps = ps_t.tile([128, 512], F32, tag="t")
                for k3 in range(3):
                    nc.tensor.matmul(bc_ps[0:1, 0:C], _r(bq_sb[:, k3, :]),
                                     _r(kT[:, k3, :]), start=(k3 == 0), stop=(k3 == 2))
                bias_c = mid.tile([1, C], F32R, tag="bias_c")
                nc.scalar.copy(bias_c[:], bc_ps[0:1, 0:C])

            if not has_bq:
                bias_c = None

            # vW = v @ Wp2  [64, 768]
            vw = mid.tile([C, D], F32R, tag="vw")
            for ng, (n0, nn) in enumerate(((0, 512), (512, 256))):
                vw_ps = ps_mm.tile([128, 512], F32, tag="mm")
                for k3 in range(3):
                    nc.tensor.matmul(vw_ps[0:C, 0:nn], _r(vT[:, k3, :]),
                                     _r(wp2_sb[:, k3, n0:n0 + nn]),
                                     start=(k3 == 0), stop=(k3 == 2))
                nc.scalar.copy(vw[:, n0:n0 + nn], vw_ps[0:C, 0:nn])

            return {"wqk": wqk, "vw": vw,
                    "bias_c": bias_c}

        def pass2_tile(b, t, mt):
            wqk, vw, bias_c = mt["wqk"], mt["vw"], mt["bias_c"]
            if True:
                xt = load_xt(b, t)
                sc_ps = ps_t.tile([128, 512], F32, tag="t")
                for k in range(6):
                    nc.tensor.matmul(sc_ps[:, 0:C], _r(xt[:, k, :]), _r(wqk[:, k, :]),
                                     start=(k == 0), stop=(k == 5 and not has_bq),
                                     skip_group_check=True)
                if has_bq:
                    nc.tensor.matmul(sc_ps[:, 0:C], _r(ones_sb[:]), _r(bias_c[:]),
                                     start=False, stop=True, skip_group_check=True)

                e_att = p_sm.tile([128, C], F32, tag="e_att")
                ssum_a = p_sm.tile([128, 1], F32, tag="ssum_a")
                nc.scalar.activation(e_att[:], sc_ps[:, 0:C], AF.Exp,
                                     scale=inv_sqrt_p, accum_out=ssum_a[:])
                rs_a = p_sm.tile([128, 1], F32, tag="rs_a")
                nc.vector.reciprocal(rs_a[:], ssum_a[:])
                attn = p_sm.tile([128, C], F32R, tag="attn")
                nc.vector.tensor_scalar_mul(attn[:], e_att[:], rs_a[:])

                at_ps = ps_t.tile([128, 512], F32R, tag="t")
                transpose_to(at_ps[0:C, 0:128], attn[:])
                attnT = p_sm.tile([C, 128], F32R, tag="attnT")
                nc.vector.tensor_copy(attnT[:], _f(at_ps[0:C, 0:128]))

                outt = p_out.tile([128, D], F32, tag="outt")
                for ng, (n0, nn) in enumerate(((0, 512), (512, 256))):
                    fo_ps = ps_mm.tile([128, 512], F32, tag="mm")
                    nc.tensor.matmul(fo_ps[:, 0:nn], _r(attnT[:]),
                                     _r(vw[:, n0:n0 + nn]), start=True, stop=True)
                    if has_bp2:
                        nc.vector.tensor_add(outt[:, n0:n0 + nn], fo_ps[:, 0:nn],
                                             bp2_sb[:, n0:n0 + nn])
                    else:
                        nc.scalar.copy(outt[:, n0:n0 + nn], fo_ps[:, 0:nn])
                nc.sync.dma_start(out_d[b, t * 128:(t + 1) * 128, :], outt[:])

        # ---------------- pipelined emission ----------------
        # pass1 runs with a 2-tile software skew (act mms of tile t overlap
        # fea mms/transposes of tile t+2); mid+pass2 of the previous batch
        # fill the pipeline slots so the PE never idles past the HAM window.
        def pass1_batch(b, slots):
            si = 0
            seg_X(b, 0); seg_F(b, 0); seg_T(b, 0)
            seg_X(b, 1); seg_F(b, 1); seg_T(b, 1)
            for t in range(NT):
                if t + 2 < NT:
                    seg_X(b, t + 2)
                seg_A(b, t)
                if t + 2 < NT:
                    seg_F(b, t + 2)
                seg_C(b, t)
                if t + 2 < NT:
                    seg_T(b, t + 2)
                if si < len(slots):
                    slots[si](); si += 1
            for f in slots[si:]:
                f()

        pass1_batch(0, [])
        for b in range(NB):
            mt_box = {}
            def do_mid(b=b, box=mt_box):
                box["mt"] = mid_phase(b)
            p2 = [do_mid] + [
                (lambda b=b, t=t, box=mt_box: pass2_tile(b, t, box["mt"]))
                for t in range(NT)]
            if b + 1 < NB:
                pass1_batch(b + 1, p2)
            else:
                for f in p2:
                    f()

    nc.compile()
    return nc


_PROGRAM_CACHE = {}


def _prep(inputs):
    """Host-side folds. Returns (flags, in_map_common)."""
    f32 = np.float32
    g = {k: np.ascontiguousarray(np.asarray(v, dtype=f32)) for k, v in inputs.items()}
    s1 = g["bn1_g"] / np.sqrt(g["bn1_v"] + f32(EPS))
    cwf = np.concatenate([g["cluster_weights"] * s1[None, :], g["W_ga"]], axis=1)
    bias1 = np.concatenate([g["bn1_b"] - g["bn1_m"] * s1, g["b_ga"]])
    s2 = g["bn2_g"] / np.sqrt(g["bn2_v"] + f32(EPS))
    bias2 = (g["b_proj"][None, :] - g["bn2_m"][:, None]) * s2[:, None] + g["bn2_b"][:, None]
    flags = (
        bool(np.any(g["b_exp"])),
        bool(np.any(g["bq"])),
        bool(np.any(g["bkv"])),
        bool(np.any(g["bp2"])),
    )
    common = {
        "wexp": g["W_exp"],
        "cwf": np.ascontiguousarray(cwf),
        "bias1": np.ascontiguousarray(bias1),
        "wproj": g["W_proj"],
        "s2": np.ascontiguousarray(s2.reshape(C, 1)),
        "bias2": np.ascontiguousarray(bias2),
        "wkv": g["Wkv"],
        "wqT": np.ascontiguousarray(g["Wq"].T),
        "wp2": g["Wp2"],
    }
    if flags[0]:
        common["bexp"] = g["b_exp"]
    if flags[1]:
        common["bqT"] = np.ascontiguousarray(g["bq"].reshape(P, 1))
    if flags[2]:
        common["bkv"] = g["bkv"]
    if flags[3]:
        common["bp2"] = g["bp2"]
    return flags, common, g["x"]


def run(inputs, trace=False):
    flags, common, x = _prep(inputs)
    if flags not in _PROGRAM_CACHE:
        _PROGRAM_CACHE[flags] = build_program(flags)
    nc = _PROGRAM_CACHE[flags]
    in_maps = []
    for c in range(N_CORES):
        m = dict(common)
        m["x"] = np.ascontiguousarray(x[c * NB:(c + 1) * NB])
        in_maps.append(m)
    res = bass_utils.run_bass_kernel_spmd(
        nc, in_maps, core_ids=list(range(N_CORES)), trace=trace)
    out = np.concatenate([r["out"] for r in res.results], axis=0)
    return out, res


def kernel(**inputs):
    out, _ = run(inputs, trace=False)
    return out



# revision 12
# speedup vs baseline: 15.7759x; 15.7759x over previous
"""ClusterAttn Trainium2 kernel (Bass/Tile), 8-way data parallel over batch.

Full inputs in, full outputs out. Internally:
  - batch B=32 is split 4-per-core across 8 NeuronCores (pure DP).
  - all PE compute in bf16 (fp32 psum accumulate); host casts inputs.
  - act/gate GEMM folded: Wcomb = W_exp @ [BN1-folded cluster_weights | W_ga]
    contracts over D=768 straight from the transposed-x tiles, so fea never
    needs a PE transpose and the cluster GEMM halves.
  - x is pre-transposed on the host to [b, ko, ki, s] so each batch loads
    with ONE fully-contiguous-per-partition DMA; weights likewise arrive in
    their SBUF layout (one descriptor row per partition). Output is staged
    per batch in SBUF and stored with one DMA per batch.
  - attention re-associated: scores = x @ (Wq @ k^T), out = attn @ (v @ Wp2).
"""

from contextlib import ExitStack

import numpy as np
import ml_dtypes

import concourse.bass as bass
import concourse.bacc as bacc
import concourse.tile as tile
import concourse.mybir as mybir
from concourse import bass_utils
from concourse.masks import make_identity

dt = mybir.dt
AF = mybir.ActivationFunctionType
ALU = mybir.AluOpType

EPS = 1e-5
N_CORES = 8
B, S, D = 32, 1024, 768
E, G, C, P = 2, 8, 64, 384
EF = E * D            # 1536
GC = G * C            # 512
GCG = GC + G          # 520
GFS = EF // G         # 192
NB = B // N_CORES     # batches per core
NT = S // 128         # token tiles per batch
KD = D // 128         # 6 contraction k-tiles over D
F32 = dt.float32
BF16 = dt.bfloat16
BF16NP = ml_dtypes.bfloat16


def build_program(flags):
    has_bexp, has_bq, has_bkv, has_bp2 = flags
    nc = bacc.Bacc(
        "TRN2",
        debug=False,
        enable_asserts=False,
        num_devices=N_CORES,
    )

    # x pre-transposed on host: xt[b, ko, ki, s] = x[b, s, ko*128+ki]
    xt_d = nc.dram_tensor("xt", (NB, KD, 128, S), BF16, kind="ExternalInput").ap()
    out_d = nc.dram_tensor("out", (NB, S, D), F32, kind="ExternalOutput").ap()
    # weights already in SBUF layout [ki, ko, n]
    wexp_d = nc.dram_tensor("wexp", (128, KD, EF), BF16, kind="ExternalInput").ap()
    wcomb_d = nc.dram_tensor("wcomb", (128, KD, GCG), BF16, kind="ExternalInput").ap()
    bias1_d = nc.dram_tensor("bias1", (GCG,), F32, kind="ExternalInput").ap()
    wproj_d = nc.dram_tensor("wproj", (128, 2, D), BF16, kind="ExternalInput").ap()
    s2_d = nc.dram_tensor("s2", (C, 1), F32, kind="ExternalInput").ap()
    bias2_d = nc.dram_tensor("bias2", (C, D), F32, kind="ExternalInput").ap()
    wkv_d = nc.dram_tensor("wkv", (128, KD, 2 * P), BF16, kind="ExternalInput").ap()
    wqT_d = nc.dram_tensor("wqT", (128, 3, D), BF16, kind="ExternalInput").ap()
    wp2_d = nc.dram_tensor("wp2", (128, 3, D), BF16, kind="ExternalInput").ap()
    bexp_d = bq_d = bkv_d = bp2_d = None
    if has_bexp:
        bexp_d = nc.dram_tensor("bexp", (EF,), F32, kind="ExternalInput").ap()
    if has_bq:
        bq_d = nc.dram_tensor("bqT", (128, 3, 1), BF16, kind="ExternalInput").ap()
    if has_bkv:
        bkv_d = nc.dram_tensor("bkv", (2 * P,), F32, kind="ExternalInput").ap()
    if has_bp2:
        bp2_d = nc.dram_tensor("bp2", (D,), F32, kind="ExternalInput").ap()

    with tile.TileContext(nc) as tc, ExitStack() as ctx:
        # ---------------- pools ----------------
        const = ctx.enter_context(tc.tile_pool(name="const", bufs=1))
        mid = ctx.enter_context(tc.tile_pool(name="mid", bufs=1))
        p_xt = ctx.enter_context(tc.tile_pool(name="p_xt", bufs=3))
        p_fea = ctx.enter_context(tc.tile_pool(name="p_fea", bufs=3))
        p_act = ctx.enter_context(tc.tile_pool(name="p_act", bufs=2))
        p_sm = ctx.enter_context(tc.tile_pool(name="p_sm", bufs=4))
        p_out = ctx.enter_context(tc.tile_pool(name="p_out", bufs=2))
        p_cacc = ctx.enter_context(tc.tile_pool(name="p_cacc", bufs=2))
        ps_mm = ctx.enter_context(tc.tile_pool(name="ps_mm", bufs=4, space="PSUM"))
        ps_t = ctx.enter_context(tc.tile_pool(name="ps_t", bufs=2, space="PSUM"))
        ps_c = ctx.enter_context(tc.tile_pool(name="ps_c", bufs=2, space="PSUM"))

        # ---------------- x prefetch + constants ----------------
        x_store = {}
        cent_store = {}

        def load_xt_batch(b):
            xt = p_xt.tile([128, KD, S], BF16, tag="xt", name=f"xt{b}")
            nc.sync.dma_start(xt[:], xt_d[b].rearrange("ko ki s -> ki ko s"))
            x_store[b] = xt
            cent_store[b] = ps_c.tile([128, 384], F32, tag="cent", name=f"centps{b}")
            return xt

        load_xt_batch(0)

        ident = const.tile([128, 128], BF16)
        ident_f = const.tile([128, 128], F32)
        make_identity(nc, ident_f[:])
        nc.vector.tensor_copy(ident[:], ident_f[:])

        wexp_sb = const.tile([128, KD, EF], BF16)
        nc.sync.dma_start(wexp_sb[:], wexp_d)
        wcomb_sb = const.tile([128, KD, GCG], BF16)
        nc.sync.dma_start(wcomb_sb[:], wcomb_d)
        bias1_sb = const.tile([128, GCG], F32)
        nc.gpsimd.dma_start(bias1_sb[:], bias1_d.partition_broadcast(128))
        wproj_sb = const.tile([128, 2, D], BF16)
        nc.sync.dma_start(wproj_sb[:], wproj_d)
        s2_sb = const.tile([C, 1], F32)
        nc.sync.dma_start(s2_sb[:], s2_d)
        bias2_sb = const.tile([C, D], F32)
        nc.sync.dma_start(bias2_sb[:], bias2_d)
        wkv_sb = const.tile([128, KD, 2 * P], BF16)
        nc.sync.dma_start(wkv_sb[:], wkv_d)
        wqT_sb = const.tile([128, 3, D], BF16)
        nc.sync.dma_start(wqT_sb[:], wqT_d)
        wp2_sb = const.tile([128, 3, D], BF16)
        nc.sync.dma_start(wp2_sb[:], wp2_d)
        if has_bexp:
            bexp_sb = const.tile([128, EF], F32)
            nc.gpsimd.dma_start(bexp_sb[:], bexp_d.partition_broadcast(128))
        if has_bq:
            bq_sb = const.tile([128, 3, 1], BF16)
            nc.sync.dma_start(bq_sb[:], bq_d)
            ones_sb = const.tile([1, 128], BF16)
            nc.vector.memset(ones_sb[:], 1.0)
        if has_bkv:
            bkv_sb = const.tile([C, 2 * P], F32)
            nc.gpsimd.dma_start(bkv_sb[:], bkv_d.partition_broadcast(C))
        if has_bp2:
            bp2_sb = const.tile([128, D], F32)
            nc.gpsimd.dma_start(bp2_sb[:], bp2_d.partition_broadcast(128))

        def transpose_to(out_ps, in_ap, start=True, stop=True):
            """PE transpose of bf16 in_ap -> fp32 psum tile slice."""
            kp = in_ap.partition_size()
            nc.tensor.matmul(
                out_ps,
                in_ap,
                ident[0:kp, 0:kp],
                is_transpose=True,
                start=start,
                stop=stop,
                skip_group_check=True,
            )

        seg = {}
        inv_sqrt_p = float(1.0 / np.sqrt(np.float32(P)))

        def seg_F(b, t):
            """fea GEMM + act/gate GEMM, all from xt; fea copies to SBUF."""
            xt = x_store[b]
            xk = lambda k: xt[:, k, t * 128:(t + 1) * 128]
            fea = p_fea.tile([128, EF], BF16, tag="fea")
            for n3 in range(3):
                fp = ps_mm.tile([128, 512], F32, tag="mm")
                for k in range(KD):
                    nc.tensor.matmul(
                        fp[:], xk(k),
                        wexp_sb[:, k, n3 * 512:(n3 + 1) * 512],
                        start=(k == 0), stop=(k == KD - 1))
                dst = fea[:, n3 * 512:(n3 + 1) * 512]
                if has_bexp:
                    nc.vector.tensor_add(dst, fp[:],
                                         bexp_sb[:, n3 * 512:(n3 + 1) * 512])
                else:
                    nc.scalar.copy(dst, fp[:])
            seg[(b, t)] = {"fea": fea}
            aps = []
            for a0, an in ((0, 256), (256, 264)):
                apm = ps_mm.tile([128, 264], F32, tag="mm")
                for k in range(KD):
                    nc.tensor.matmul(
                        apm[:, 0:an], xk(k),
                        wcomb_sb[:, k, a0:a0 + an],
                        start=(k == 0), stop=(k == KD - 1))
                aps.append((apm, a0, an))
            seg[(b, t)]["aps"] = aps

        def seg_S(b, t):
            """grouped softmax * sigmoid gate -> actf (bf16)."""
            st = seg[(b, t)]
            act = p_act.tile([128, GCG], F32, tag="act")
            for apm, a0, an in st.pop("aps"):
                nc.vector.tensor_add(act[:, a0:a0 + an], apm[:, 0:an],
                                     bias1_sb[:, a0:a0 + an])
            e = p_act.tile([128, GC], F32, tag="e")
            nc.scalar.activation(e[:], act[:, 0:GC], AF.Exp)
            ssum = p_sm.tile([128, G], F32, tag="ssum")
            nc.vector.reduce_sum(ssum[:], e[:].rearrange("p (g c) -> p g c", g=G),
                                 axis=mybir.AxisListType.X)
            eneg = p_sm.tile([128, G], F32, tag="eneg")
            nc.scalar.activation(eneg[:], act[:, GC:GCG], AF.Exp, scale=-1.0)
            nc.vector.tensor_scalar_add(eneg[:], eneg[:], 1.0)
            ga = p_sm.tile([128, G], F32, tag="ga")
            nc.vector.reciprocal(ga[:], eneg[:])
            rs = p_sm.tile([128, G], F32, tag="rs")
            nc.vector.reciprocal(rs[:], ssum[:])
            nc.vector.tensor_mul(rs[:], rs[:], ga[:])
            actf = p_act.tile([128, GC], BF16, tag="actf")
            nc.vector.tensor_tensor(
                out=actf[:].rearrange("p (g c) -> p g c", g=G),
                in0=e[:].rearrange("p (g c) -> p g c", g=G),
                in1=rs[:].unsqueeze(2).broadcast_to((128, G, C)),
                op=ALU.mult)
            st["actf"] = actf

        def seg_C(b, t):
            st = seg.pop((b, t))
            fea, actf = st["fea"], st["actf"]
            cp = cent_store[b]
            # all 4 diagonal blocks accumulate into ONE psum bank; the two
            # extracted sub-blocks never overlap, so psum accumulation
            # performs the over-block reduction for free.
            for mi in range(4):
                nc.tensor.matmul(
                    cp[:], actf[:, mi * 128:(mi + 1) * 128],
                    fea[:, mi * 384:(mi + 1) * 384],
                    start=(t == 0 and mi == 0),
                    stop=(t == NT - 1 and mi == 3),
                    skip_group_check=True)

        def make_items(b):
            """Pipelined mid-phase chunks + pass2 a/b halves for batch b.
            Each chunk's PE work depends only on DVE/ACT output emitted at
            least one pass1-tile earlier, so the PE FIFO never blocks."""
            mt = {}

            def mid_cent():
                cp = cent_store.pop(b)
                cent = p_cacc.tile([C, GFS], BF16, tag="cacc")
                tmp = p_cacc.tile([C, GFS], F32, tag="ctmp")
                nc.scalar.copy(tmp[:], cp[0:64, 0:192])
                nc.vector.tensor_add(cent[:], tmp[:], cp[64:128, 192:384])
                mt["cent"] = cent

            def mid1():
                cent = mt["cent"]
                centT = mid.tile([128, 2, C], BF16, tag="centT")
                ctp = ps_t.tile([128, 512], BF16, tag="t")
                transpose_to(ctp[:, 0:64], cent[:, 0:128])
                transpose_to(ctp[0:64, 64:128], cent[:, 128:192])
                nc.vector.tensor_copy(centT[:, 0, :], ctp[:, 0:64])
                nc.vector.tensor_copy(centT[0:64, 1, :], ctp[0:64, 64:128])
                # nc2 = BN2(cent @ W_proj + b_proj)  [64, 768]
                nc2 = mid.tile([C, D], BF16, tag="nc2")
                for n0, nn in ((0, 512), (512, 256)):
                    np_ps = ps_mm.tile([128, 512], F32, tag="mm")
                    nc.tensor.matmul(np_ps[0:C, 0:nn], centT[:, 0, :],
                                     wproj_sb[:, 0, n0:n0 + nn], start=True, stop=False)
                    nc.tensor.matmul(np_ps[0:C, 0:nn], centT[0:64, 1, :],
                                     wproj_sb[0:64, 1, n0:n0 + nn], start=False, stop=True)
                    nc.vector.scalar_tensor_tensor(
                        out=nc2[:, n0:n0 + nn], in0=np_ps[0:C, 0:nn], scalar=s2_sb[:, 0:1],
                        in1=bias2_sb[:, n0:n0 + nn], op0=ALU.mult, op1=ALU.add)
                mt["nc2"] = nc2

            def mid2():
                nc2 = mt["nc2"]
                nc2T = mid.tile([128, KD, C], BF16, tag="nc2T")
                for grp in range(2):
                    ntp = ps_t.tile([128, 512], BF16, tag="t")
                    for i in range(3):
                        transpose_to(ntp[:, i * 64:(i + 1) * 64],
                                     nc2[:, (grp * 3 + i) * 128:(grp * 3 + i + 1) * 128],
                                     start=(i == 0), stop=(i == 2))
                    nc.vector.tensor_copy(
                        nc2T[:, grp * 3:(grp + 1) * 3, :].rearrange("p a b -> p (a b)"),
                        ntp[:, 0:192])
                kv = mid.tile([C, 2 * P], BF16, tag="kv")
                for n0, nn in ((0, 512), (512, 256)):
                    kv_ps = ps_mm.tile([128, 512], F32, tag="mm")
                    for k in range(KD):
                        nc.tensor.matmul(kv_ps[0:C, 0:nn], nc2T[:, k, :],
                                         wkv_sb[:, k, n0:n0 + nn],
                                         start=(k == 0), stop=(k == KD - 1))
                    if has_bkv:
                        nc.vector.tensor_add(kv[:, n0:n0 + nn], kv_ps[0:C, 0:nn],
                                             bkv_sb[:, n0:n0 + nn])
                    else:
                        nc.scalar.copy(kv[:, n0:n0 + nn], kv_ps[0:C, 0:nn])
                mt["kv"] = kv

            def mid3():
                kv = mt["kv"]
                kT = mid.tile([128, 3, C], BF16, tag="kT")
                vT = mid.tile([128, 3, C], BF16, tag="vT")
                ktp = ps_t.tile([128, 512], BF16, tag="t")
                for i in range(3):
                    transpose_to(ktp[:, i * 64:(i + 1) * 64], kv[:, i * 128:(i + 1) * 128],
                                 start=(i == 0), stop=(i == 2))
                nc.vector.tensor_copy(kT[:].rearrange("p a b -> p (a b)"), ktp[:, 0:192])
                vtp = ps_t.tile([128, 512], BF16, tag="t")
                for i in range(3):
                    transpose_to(vtp[:, i * 64:(i + 1) * 64],
                                 kv[:, P + i * 128:P + (i + 1) * 128],
                                 start=(i == 0), stop=(i == 2))
                nc.vector.tensor_copy(vT[:].rearrange("p a b -> p (a b)"), vtp[:, 0:192])
                mt["kT"], mt["vT"] = kT, vT

            def mid4():
                kT, vT = mt["kT"], mt["vT"]
                wqk = mid.tile([128, KD, C], BF16, tag="wqk")
                for m in range(KD):
                    wq_ps = ps_t.tile([128, 512], F32, tag="t")
                    for k3 in range(3):
                        nc.tensor.matmul(wq_ps[:, 0:C], wqT_sb[:, k3, m * 128:(m + 1) * 128],
                                         kT[:, k3, :], start=(k3 == 0), stop=(k3 == 2))
                    nc.scalar.copy(wqk[:, m, :], wq_ps[:, 0:C])
                mt["bias_c"] = None
                if has_bq:
                    bc_ps = ps_t.tile([128, 512], F32, tag="t")
                    for k3 in range(3):
                        nc.tensor.matmul(bc_ps[0:1, 0:C], bq_sb[:, k3, :],
                                         kT[:, k3, :], start=(k3 == 0), stop=(k3 == 2))
                    bias_c = mid.tile([1, C], BF16, tag="bias_c")
                    nc.scalar.copy(bias_c[:], bc_ps[0:1, 0:C])
                    mt["bias_c"] = bias_c
                vw = mid.tile([C, D], BF16, tag="vw")
                for n0, nn in ((0, 512), (512, 256)):
                    vw_ps = ps_mm.tile([128, 512], F32, tag="mm")
                    for k3 in range(3):
                        nc.tensor.matmul(vw_ps[0:C, 0:nn], vT[:, k3, :],
                                         wp2_sb[:, k3, n0:n0 + nn],
                                         start=(k3 == 0), stop=(k3 == 2))
                    nc.scalar.copy(vw[:, n0:n0 + nn], vw_ps[0:C, 0:nn])
                mt["wqk"], mt["vw"] = wqk, vw
                mt["out_sb"] = p_out.tile([128, NT, D], F32, tag="out", name=f"out{b}")

            def p2a(t):
                """scores + softmax -> attn(t); PE part is tiny (N=64 MMs)."""
                wqk, bias_c = mt["wqk"], mt["bias_c"]
                xt = x_store[b]
                sc_ps = ps_t.tile([128, 512], F32, tag="t")
                for k in range(KD):
                    nc.tensor.matmul(sc_ps[:, 0:C],
                                     xt[:, k, t * 128:(t + 1) * 128], wqk[:, k, :],
                                     start=(k == 0), stop=(k == KD - 1 and not has_bq),
                                     skip_group_check=True)
                if has_bq:
                    nc.tensor.matmul(sc_ps[:, 0:C], ones_sb[:], bias_c[:],
                                     start=False, stop=True, skip_group_check=True)
                e_att = p_sm.tile([128, C], F32, tag="e_att")
                ssum_a = p_sm.tile([128, 1], F32, tag="ssum_a")
                nc.scalar.activation(e_att[:], sc_ps[:, 0:C], AF.Exp,
                                     scale=inv_sqrt_p, accum_out=ssum_a[:])
                rs_a = p_sm.tile([128, 1], F32, tag="rs_a")
                nc.vector.reciprocal(rs_a[:], ssum_a[:])
                attn = p_sm.tile([128, C], BF16, tag="attn")
                nc.vector.tensor_scalar_mul(attn[:], e_att[:], rs_a[:])
                mt[("attn", t)] = attn

            def p2b(t):
                """attn transpose + out GEMM + out staging; runs >=1 slot
                after p2a(t) so attn is ready when the PE reaches it."""
                vw, out_sb = mt["vw"], mt["out_sb"]
                attn = mt.pop(("attn", t))
                at_ps = ps_t.tile([128, 512], BF16, tag="t")
                transpose_to(at_ps[0:C, 0:128], attn[:])
                attnT = p_sm.tile([C, 128], BF16, tag="attnT")
                nc.vector.tensor_copy(attnT[:], at_ps[0:C, 0:128])
                for n0, nn in ((0, 512), (512, 256)):
                    fo_ps = ps_mm.tile([128, 512], F32, tag="mm")
                    nc.tensor.matmul(fo_ps[:, 0:nn], attnT[:],
                                     vw[:, n0:n0 + nn], start=True, stop=True)
                    if has_bp2:
                        nc.vector.tensor_add(out_sb[:, t, n0:n0 + nn], fo_ps[:, 0:nn],
                                             bp2_sb[:, n0:n0 + nn])
                    else:
                        nc.scalar.copy(out_sb[:, t, n0:n0 + nn], fo_ps[:, 0:nn])
                half = NT // 2
                if t == half - 1 or t == NT - 1:
                    h0 = t + 1 - half
                    nc.gpsimd.dma_start(
                        out_d[b].rearrange("(t p) d -> p t d", p=128)[:, h0:t + 1, :],
                        out_sb[:, h0:t + 1, :])
                if t == NT - 1:
                    x_store.pop(b)

            # slot items: mid chunks then a/b halves with one-slot skew.
            # boundary runs in the NEXT batch's pre; late merges into its
            # first slot so p2b(7)'s softmax input has a pass1 tile of slack.
            slots = [
                [mid1], [mid2], [mid3],
                [mid4, lambda: p2a(0)],
                [lambda: p2a(1), lambda: p2b(0)],
                [lambda: p2a(2), lambda: p2b(1)],
                [lambda: p2a(3), lambda: p2b(2)],
                [lambda: p2a(4), lambda: p2b(3)],
            ]
            boundary = [lambda: p2a(5), lambda: p2b(4), lambda: p2a(6)]
            late = [lambda: p2b(5), lambda: p2a(7)]
            late2 = [lambda: p2b(6), lambda: p2b(7)]
            return [mid_cent], slots, boundary, late, late2

        # ---------------- pipelined emission ----------------
        # 1-tile software skew: softmax (DVE/ACT) of tile t overlaps the
        # fea/act GEMMs of tile t+1; cent of t lands after F(t+1) on the PE
        # queue. mid+pass2 of the previous batch fill the slots.
        def pass1_batch(b, pre, slots, post):
            for f in pre:
                f()
            si = 0
            for t in range(NT):
                seg_F(b, t)
                seg_S(b, t)
                if t > 0:
                    seg_C(b, t - 1)
                if si < len(slots):
                    for f in slots[si]:
                        f()
                    si += 1
            seg_C(b, NT - 1)
            for group in slots[si:]:
                for f in group:
                    f()
            for group in post:
                for f in group:
                    f()

        pass1_batch(0, [lambda: load_xt_batch(1)], [], [])
        carry_boundary, carry_late, carry_late2 = [], [], []
        for b in range(NB):
            pre, slots, boundary, late, late2 = make_items(b)
            if b + 1 < NB:
                pre2 = list(pre) + carry_boundary
                if b + 2 < NB:
                    pre2.append(lambda bb=b + 2: load_xt_batch(bb))
                slots2 = ([carry_late + slots[0], carry_late2 + slots[1]]
                          + slots[2:])
                pass1_batch(b + 1, pre2, slots2, [])
                carry_boundary, carry_late, carry_late2 = boundary, late, late2
            else:
                for f in carry_boundary + carry_late + carry_late2 + pre:
                    f()
                for group in slots:
                    for f in group:
                        f()
                for f in boundary + late + late2:
                    f()

    nc.compile()
    return nc


_PROGRAM_CACHE = {}


def _prep(inputs):
    """Host-side folds + layout packing. Returns (flags, common, xt_bf16)."""
    f32 = np.float32
    g = {k: np.ascontiguousarray(np.asarray(v, dtype=f32)) for k, v in inputs.items()}
    s1 = g["bn1_g"] / np.sqrt(g["bn1_v"] + f32(EPS))
    cwf = np.concatenate([g["cluster_weights"] * s1[None, :], g["W_ga"]], axis=1)
    bias1 = np.concatenate([g["bn1_b"] - g["bn1_m"] * s1, g["b_ga"]]) + g["b_exp"] @ cwf
    wcomb = g["W_exp"] @ cwf
    s2 = g["bn2_g"] / np.sqrt(g["bn2_v"] + f32(EPS))
    bias2 = (g["b_proj"][None, :] - g["bn2_m"][:, None]) * s2[:, None] + g["bn2_b"][:, None]
    flags = (
        bool(np.any(g["b_exp"])),
        bool(np.any(g["bq"])),
        bool(np.any(g["bkv"])),
        bool(np.any(g["bp2"])),
    )

    def ki_ko(w):
        """(ko*128+ki, n) fp32 -> bf16 [128 ki, ko, n] SBUF layout."""
        ko = w.shape[0] // 128
        return np.ascontiguousarray(
            w.reshape(ko, 128, w.shape[1]).transpose(1, 0, 2).astype(BF16NP))

    wproj_p = np.zeros((2, 128, D), f32)
    wproj_p[0] = g["W_proj"][0:128]
    wproj_p[1, 0:64] = g["W_proj"][128:GFS]
    common = {
        "wexp": ki_ko(g["W_exp"]),
        "wcomb": ki_ko(wcomb),
        "bias1": np.ascontiguousarray(bias1.astype(f32)),
        "wproj": np.ascontiguousarray(wproj_p.transpose(1, 0, 2).astype(BF16NP)),
        "s2": np.ascontiguousarray(s2.reshape(C, 1)),
        "bias2": np.ascontiguousarray(bias2),
        "wkv": ki_ko(g["Wkv"]),
        "wqT": ki_ko(np.ascontiguousarray(g["Wq"].T)),
        "wp2": ki_ko(g["Wp2"]),
    }
    if flags[0]:
        common["bexp"] = g["b_exp"]
    if flags[1]:
        common["bqT"] = ki_ko(g["bq"].reshape(P, 1))
    if flags[2]:
        common["bkv"] = g["bkv"]
    if flags[3]:
        common["bp2"] = g["bp2"]
    # x -> [b, ko, ki, s] bf16
    xt = np.ascontiguousarray(
        g["x"].reshape(B, S, KD, 128).transpose(0, 2, 3, 1).astype(BF16NP))
    return flags, common, xt


def run(inputs, trace=False):
    flags, common, xt = _prep(inputs)
    if flags not in _PROGRAM_CACHE:
        _PROGRAM_CACHE[flags] = build_program(flags)
    nc = _PROGRAM_CACHE[flags]
    in_maps = []
    for c in range(N_CORES):
        m = dict(common)
        m["xt"] = np.ascontiguousarray(xt[c * NB:(c + 1) * NB])
        in_maps.append(m)
    res = bass_utils.run_bass_kernel_spmd(
        nc, in_maps, core_ids=list(range(N_CORES)), trace=trace)
    out = np.concatenate([r["out"] for r in res.results], axis=0)
    return out, res


def kernel(**inputs):
    out, _ = run(inputs, trace=False)
    return out
